# revision 6
# baseline (speedup 1.0000x reference)
"""Trainium2 Bass kernel for nn_DirectionalContrastiveLoss (8-core SPMD).

Algorithmic structure
---------------------
The reference loss is  mean over masked rows of  -log(lg + 1e-8)  with
lg = exp(pos - M) / (S_masked + 1e-8),  M = row max over [pos, scores].
Because of the 1e-8 clamp inside the log, any row whose pos is more than
~43 below its row max contributes exactly -log(1e-8): lg <= e^{pos-M}*1e8
is then < 2e-11 and shifts the log by < 1e-2 * 1e-8. With randn features
the scores have std ~160 and row maxes ~600, so only a handful of rows in
the whole problem deviate from the clamp.

The device therefore only needs (a) the score matmul for the masked rows
(25.6% of rows appear in the loss at all) and (b) a per-row approximate
max. A subsampled max can only UNDER-estimate, which only widens the
host-side selection - never corrupts it. The few selected rows (plus any
rows beyond the device tile capacity) get an exact float64 masked softmax
on the host; every other masked row contributes the clamp constant.

Device kernel per core: fp8 DoubleRow matmuls (both 128-deep k-tiles of
the K=256 contraction in one pass) of [128-row tile] x [8000-col bank,
replicated, order-free] accumulating fp32 in PSUM, then a stride-4
reduce_max per 1024-col fill. No exp / softmax machinery on device.

Performance notes (vs the v1 of this design):
- bank DRAM/SBUF layout is [128, N, 2] so a column-chunk DMA moves
  2*width contiguous bytes per partition (fewer, larger descriptors);
- chunk DMAs are spread over the three DMA initiators (sync HWDGE,
  scalar HWDGE, gpsimd SWDGE) so rings run in parallel;
- dummy matmuls on a memset tile run during the DMA head to lift the
  PE HAM clock gate (cold 1.2 GHz -> warm 2.4 GHz) before real work.
"""
import numpy as np
import ml_dtypes

import bass_rust
import concourse.bass as bass
import concourse.tile as tile
from concourse import mybir
from concourse.bass_utils import run_bass_kernel_spmd
from concourse.vector_clock import ScopedClock

F8 = ml_dtypes.float8_e4m3   # TRN fp8e4: max normal +-240
N_CORES = 8
TEMP = 0.1
POS_THRESH = 0.7
EPS = 1e-8
N = 8000          # anchors (== memory slots)
C = 256           # feature channels
FILL = 1024       # PSUM fill width (2 banks of fp32)
SUB = 4           # reduce_max column subsample stride
MARGIN = 110.0    # selection threshold slack (clamp 43 + fp8 max-error tail)
HOST_CAP = 64     # max rows/direction computed on host due to capacity
COLSTEP = 1       # device column subsample (1 = all bank columns)

LAST_RESULTS = None  # BassKernelResults of the most recent kernel() call

# ---------------------------------------------------------------------------
# walrus in this toolchain rejects >1 sync wait per instruction; spread the
# TileContext tail-drain waits over single-wait sync NOPs.
_N_SPILL_NOPS = 24


def _patched_drain_and_barrier(self, tick_clock, wait_clock):
    nops = [self.nc.sync.nop(nofuse=True, hint=f"drainwait{i}")
            for i in range(_N_SPILL_NOPS)]
    drain_inst = self.nc.sync.drain()
    wait_clock.add_sem_waits(drain_inst.ins,
                             ScopedClock({None: tick_clock.global_clock}))
    si = drain_inst.ins.sync_info
    waits = list(si.on_wait) if si is not None else []
    if waits:
        assert len(waits) <= _N_SPILL_NOPS
        for i, w in enumerate(waits):
            nops[i].ins.sync_info = bass_rust.SyncInfo(on_wait=[w], on_update=[])
        drain_inst.ins.sync_info = bass_rust.SyncInfo(
            on_wait=[], on_update=list(si.on_update))
    self.nc.all_engine_barrier()
    popped = self.nc._tile_sem_poison_stack.pop()
    assert popped is self._sem_poison
    self.nc.clear_and_free_semaphores(list(self.sems.allocated().values()))


tile.TileContext._drain_and_barrier = _patched_drain_and_barrier

# Same walrus limitation for regular scheduled instructions: split any
# multi-wait instruction into single-wait same-engine NOPs + the instruction
# keeping its last wait (sequential waits on one engine are equivalent).
_orig_lower_ordered = tile.TileContext._lower_ordered_insts


def _split_multiwait_lower(self, ordered):
    for bb, insts in ordered.items():
        out = []
        for inst in insts:
            si = inst.sync_info
            waits = list(si.on_wait) if si is not None else []
            if len(waits) > 1:
                for w in waits[:-1]:
                    out.append(mybir.InstNoOp(
                        name=self.nc.get_next_instruction_name(),
                        sync_info=mybir.SyncInfo(on_wait=[w], on_update=[]),
                        engine=inst.engine,
                        bass_nofuse=True,
                        text_hint="waitsplit",
                    ))
                inst.sync_info = mybir.SyncInfo(
                    on_wait=[waits[-1]], on_update=list(si.on_update))
            out.append(inst)
        ordered[bb] = out
    return _orig_lower_ordered(self, ordered)


tile.TileContext._lower_ordered_insts = _split_multiwait_lower


# ---------------------------------------------------------------------------
def _fills(ncols):
    out = []
    c = 0
    while c < ncols:
        out.append((c, min(FILL, ncols - c)))
        c += FILL
    return out


def _build_program(ntot, ncols):
    """SPMD program: ntot 128-row tiles x [ncols]-col bank matmul + max."""
    nc = bass.Bass("TRN2", target_bir_lowering=False, debug=False,
                   num_devices=N_CORES)
    f8, f32 = mybir.dt.float8e4, mybir.dt.float32
    AX = mybir.AxisListType.X
    DR = mybir.MatmulPerfMode.DoubleRow
    fills = _fills(ncols)
    nf = len(fills)

    d_bank = nc.dram_tensor("bank", [128, ncols, 2], f8,
                            kind="ExternalInput").ap()
    d_fT = nc.dram_tensor("fT", [128, 2, ntot * 128], f8,
                          kind="ExternalInput").ap()
    d_out = nc.dram_tensor("negm", [128, ntot * nf], f32,
                           kind="ExternalOutput").ap()

    with tile.TileContext(nc) as tc:
        import contextlib
        with contextlib.ExitStack() as ctx:
            singles = ctx.enter_context(tc.tile_pool(name="singles", bufs=1))
            psum = ctx.enter_context(tc.tile_pool(name="psum", bufs=3,
                                                  space="PSUM"))
            wpsum = ctx.enter_context(tc.tile_pool(name="wpsum", bufs=1,
                                                   space="PSUM"))
            bank = singles.tile([128, ncols, 2], f8, tag="bank", name="bank")
            fT = singles.tile([128, 2, ntot * 128], f8, tag="fT", name="fT")
            outm = singles.tile([128, ntot * nf], f32, tag="outm", name="outm")

            # PE warmup during the DMA head: matmuls on a memset tile keep
            # the PE busy through one HAM activity window so the clock is
            # already at 2.4 GHz when the real matmuls start.
            wsrc = singles.tile([128, 2, 256], f8, tag="wsrc", name="wsrc")
            nc.gpsimd.memset(wsrc, 0.0)
            wps = wpsum.tile([128, 128], f32, tag="wps", name="wps")
            for _ in range(16):
                nc.tensor.matmul(wps, wsrc[:, :, 0:256:2],
                                 wsrc[:, :, 0:256:2], start=True, stop=True,
                                 perf_mode=DR)

            # input DMA, spread across the three initiators' rings
            nc.scalar.dma_start(out=fT, in_=d_fT)
            CH = 1024
            chunks = [(c0, min(CH, ncols - c0)) for c0 in range(0, ncols, CH)]
            ring = {0: nc.sync, 1: nc.sync, 2: nc.sync, 3: nc.sync,
                    4: nc.scalar, 5: nc.scalar, 6: nc.gpsimd, 7: nc.gpsimd}
            for ci, (c0, w) in enumerate(chunks):
                eng = ring.get(ci, nc.sync)
                eng.dma_start(out=bank[:, c0:c0 + w, :],
                              in_=d_bank[:, c0:c0 + w, :])

            for s in range(ntot):
                lhs = fT[:, :, s * 128:(s + 1) * 128]
                for fi, (c0, w) in enumerate(fills):
                    ps = psum.tile([128, FILL], f32, tag="ps", name="ps")
                    for h0 in range(0, w, 512):
                        hw = min(512, w - h0)
                        rhs = bank[:, c0 + h0:c0 + h0 + hw, :].rearrange(
                            "p c k -> p k c")
                        nc.tensor.matmul(ps[:, h0:h0 + hw], lhs, rhs,
                                         start=True, stop=True, perf_mode=DR)
                    nc.vector.reduce_max(
                        out=outm[:, s * nf + fi:s * nf + fi + 1],
                        in_=ps[:, 0:w:SUB], axis=AX)
            nc.sync.dma_start(out=d_out, in_=outm)

    return nc


# ---------------------------------------------------------------------------
def kernel(output_feat1, output_feat2, pseudo_label1, pseudo_label2,
           pseudo_logits1, pseudo_logits2, output_ul1, output_ul2,
           selected_idx1, selected_idx2):
    f1 = np.ascontiguousarray(np.asarray(output_feat1, dtype=np.float32))
    f2 = np.ascontiguousarray(np.asarray(output_feat2, dtype=np.float32))
    pl1 = np.asarray(pseudo_label1).astype(np.int64)
    pl2 = np.asarray(pseudo_label2).astype(np.int64)
    pg1 = np.asarray(pseudo_logits1, dtype=np.float32)
    pg2 = np.asarray(pseudo_logits2, dtype=np.float32)
    ul1 = np.asarray(output_ul1, dtype=np.float32)
    ul2 = np.asarray(output_ul2, dtype=np.float32)
    idx1 = np.asarray(selected_idx1).astype(np.int64)
    idx2 = np.asarray(selected_idx2).astype(np.int64)

    b, c, h, w = ul1.shape
    ul1f = ul1.transpose(0, 2, 3, 1).reshape(-1, c)
    ul2f = ul2.transpose(0, 2, 3, 1).reshape(-1, c)
    memory = np.concatenate([ul1f[idx1], ul2f[idx2]], axis=0)     # [N, C]
    ml = np.concatenate([pl1[idx1], pl2[idx2]], axis=0)           # [N]

    pm = [((pg2 > POS_THRESH) & (pg1 < pg2)),
          ((pg1 > POS_THRESH) & (pg2 < pg1))]
    anchors = [f1, f2]
    alabels = [pl1, pl2]
    rows = [np.nonzero(m)[0] for m in pm]
    counts = [len(r) for r in rows]

    # device capacity: nt[d] 128-row tiles per core per direction; at most
    # HOST_CAP overflow rows per direction fall back to exact host compute
    nt = [max(1, -(-(max(cnt - HOST_CAP, 1)) // (128 * N_CORES)))
          for cnt in counts]
    cap = [t * 128 * N_CORES for t in nt]
    dev_rows = [r[:cp] for r, cp in zip(rows, cap)]
    host_rows = [r[cp:] for r, cp in zip(rows, cap)]
    ntot = nt[0] + nt[1]
    dev_cols = np.arange(0, N, COLSTEP)
    ncols = len(dev_cols)
    nf = len(_fills(ncols))

    # pos (score of the positive pair) for all masked rows, float64 on host
    pos_all = (f1.astype(np.float64) * f2.astype(np.float64)).sum(1) / TEMP

    # --- device inputs
    # bank [128, ncols, 2]: element (p, j, i) = channel i*128+p of column j
    bankT = memory[dev_cols].T.astype(np.float32)                 # [C, ncols]
    bank_dev = np.clip(bankT, -240, 240).reshape(2, 128, ncols)
    bank_dev = np.ascontiguousarray(bank_dev.transpose(1, 2, 0).astype(F8))
    # fT [128, 2, ntot*128]: element (p, i, r) = channel i*128+p of row r
    per_core = [t * 128 for t in nt]
    in_maps = []
    for core in range(N_CORES):
        cols = np.zeros((C, ntot * 128), dtype=np.float32)
        off = 0
        for d in range(2):
            sl = dev_rows[d][core * per_core[d]:(core + 1) * per_core[d]]
            if len(sl):
                cols[:, off:off + len(sl)] = anchors[d][sl].T / TEMP
            off += per_core[d]
        fTc = np.clip(cols, -240, 240).reshape(2, 128, ntot * 128)
        fTc = np.ascontiguousarray(fTc.transpose(1, 0, 2).astype(F8))
        in_maps.append({"bank": bank_dev, "fT": fTc})

    nc = _build_program(ntot, ncols)
    res = run_bass_kernel_spmd(nc, in_maps, list(range(N_CORES)))
    global LAST_RESULTS
    LAST_RESULTS = res

    # --- decode per-row approximate maxes
    mhat = [np.full(len(dev_rows[d]), -np.inf) for d in range(2)]
    for core in range(N_CORES):
        o = res.results[core]["negm"].astype(np.float64)   # [128, ntot*nf]
        for d in range(2):
            base_slot = 0 if d == 0 else nt[0]
            for t in range(nt[d]):
                g0 = core * per_core[d] + t * 128
                take = min(128, len(dev_rows[d]) - g0)
                if take <= 0:
                    continue
                sl = slice((base_slot + t) * nf, (base_slot + t + 1) * nf)
                mhat[d][g0:g0 + take] = o[:take, sl].max(axis=1)

    # --- host: exact contributions for selected + overflow rows, clamp rest
    CLAMP = -np.log(np.float64(EPS))
    mem64 = memory.astype(np.float64)
    total = np.float64(0)
    for d in range(2):
        sel = dev_rows[d][pos_all[dev_rows[d]] > mhat[d] - MARGIN]
        exact = np.concatenate([sel, host_rows[d]]).astype(np.int64)
        contrib = np.float64(0)
        if len(exact):
            A = anchors[d][exact].astype(np.float64)
            S = A @ mem64.T / TEMP                                # [k, N]
            pos_e = pos_all[exact]
            M = np.maximum(S.max(axis=1), pos_e)
            keep = (alabels[d][None, :] != ml[exact][:, None])
            Ssum = (np.exp(S - M[:, None]) * keep).sum(axis=1) \
                + np.exp(pos_e - M)
            lg = np.exp(pos_e - M) / (Ssum + EPS)
            contrib = (-np.log(lg + EPS)).sum()
        loss_d = (contrib + (counts[d] - len(exact)) * CLAMP) \
            / (counts[d] + 1e-12)
        total += loss_d
    return np.float32(total)


# revision 8
# speedup vs baseline: 1.3541x; 1.3541x over previous
"""Trainium2 Bass kernel for nn_DirectionalContrastiveLoss (8-core SPMD).

Algorithmic structure
---------------------
The reference loss is  mean over masked rows of  -log(lg + 1e-8)  with
lg = exp(pos - M) / (S_masked + 1e-8),  M = row max over [pos, scores].
Because of the 1e-8 clamp inside the log, any row whose pos is more than
~43 below its row max contributes exactly -log(1e-8): lg <= e^{pos-M}*1e8
is then < 2e-11 and shifts the log by < 1e-2 * 1e-8. With randn features
the scores have std ~160 and row maxes ~600, so only a handful of rows in
the whole problem deviate from the clamp.

The device therefore only needs (a) the score matmul for the masked rows
(25.6% of rows appear in the loss at all) and (b) a per-row approximate
max. A subsampled max can only UNDER-estimate, which only widens the
host-side selection - never corrupts it. The few selected rows (plus any
rows beyond the device tile capacity) get an exact float64 masked softmax
on the host; every other masked row contributes the clamp constant.

Device kernel per core: fp8 DoubleRow matmuls (both 128-deep k-tiles of
the K=256 contraction in one pass) of [128-row tile] x [8000-col bank,
replicated, order-free] accumulating fp32 in PSUM, then a stride-4
reduce_max per 1024-col fill. No exp / softmax machinery on device.

Performance notes (vs the v1 of this design):
- bank DRAM/SBUF layout is [128, N, 2] so a column-chunk DMA moves
  2*width contiguous bytes per partition (fewer, larger descriptors);
- chunk DMAs are spread over the three DMA initiators (sync HWDGE,
  scalar HWDGE, gpsimd SWDGE) so rings run in parallel;
- dummy matmuls on a memset tile run during the DMA head to lift the
  PE HAM clock gate (cold 1.2 GHz -> warm 2.4 GHz) before real work.
"""
import numpy as np
import ml_dtypes

import bass_rust
import concourse.bass as bass
import concourse.tile as tile
from concourse import mybir
from concourse.bass_utils import run_bass_kernel_spmd
from concourse.vector_clock import ScopedClock

F8 = ml_dtypes.float8_e4m3   # TRN fp8e4: max normal +-240
N_CORES = 8
TEMP = 0.1
POS_THRESH = 0.7
EPS = 1e-8
N = 8000          # anchors (== memory slots)
C = 256           # feature channels
FILL = 1024       # PSUM fill width (2 banks of fp32)
SUB = 4           # reduce_max column subsample stride
MARGIN = 110.0    # selection threshold slack (clamp 43 + fp8 max-error tail)
HOST_CAP = 64     # max rows/direction computed on host due to capacity
COLSTEP = 2       # device column subsample (1 = all bank columns)

LAST_RESULTS = None  # BassKernelResults of the most recent kernel() call

# ---------------------------------------------------------------------------
# walrus in this toolchain rejects >1 sync wait per instruction; spread the
# TileContext tail-drain waits over single-wait sync NOPs.
_N_SPILL_NOPS = 24


def _patched_drain_and_barrier(self, tick_clock, wait_clock):
    nops = [self.nc.sync.nop(nofuse=True, hint=f"drainwait{i}")
            for i in range(_N_SPILL_NOPS)]
    drain_inst = self.nc.sync.drain()
    wait_clock.add_sem_waits(drain_inst.ins,
                             ScopedClock({None: tick_clock.global_clock}))
    si = drain_inst.ins.sync_info
    waits = list(si.on_wait) if si is not None else []
    if waits:
        assert len(waits) <= _N_SPILL_NOPS
        for i, w in enumerate(waits):
            nops[i].ins.sync_info = bass_rust.SyncInfo(on_wait=[w], on_update=[])
        drain_inst.ins.sync_info = bass_rust.SyncInfo(
            on_wait=[], on_update=list(si.on_update))
    self.nc.all_engine_barrier()
    popped = self.nc._tile_sem_poison_stack.pop()
    assert popped is self._sem_poison
    self.nc.clear_and_free_semaphores(list(self.sems.allocated().values()))


tile.TileContext._drain_and_barrier = _patched_drain_and_barrier

# Same walrus limitation for regular scheduled instructions: split any
# multi-wait instruction into single-wait same-engine NOPs + the instruction
# keeping its last wait (sequential waits on one engine are equivalent).
_orig_lower_ordered = tile.TileContext._lower_ordered_insts


def _split_multiwait_lower(self, ordered):
    for bb, insts in ordered.items():
        out = []
        for inst in insts:
            si = inst.sync_info
            waits = list(si.on_wait) if si is not None else []
            if len(waits) > 1:
                for w in waits[:-1]:
                    out.append(mybir.InstNoOp(
                        name=self.nc.get_next_instruction_name(),
                        sync_info=mybir.SyncInfo(on_wait=[w], on_update=[]),
                        engine=inst.engine,
                        bass_nofuse=True,
                        text_hint="waitsplit",
                    ))
                inst.sync_info = mybir.SyncInfo(
                    on_wait=[waits[-1]], on_update=list(si.on_update))
            out.append(inst)
        ordered[bb] = out
    return _orig_lower_ordered(self, ordered)


tile.TileContext._lower_ordered_insts = _split_multiwait_lower


# ---------------------------------------------------------------------------
def _fills(ncols):
    out = []
    c = 0
    while c < ncols:
        out.append((c, min(FILL, ncols - c)))
        c += FILL
    return out


def _build_program(ntot, ncols):
    """SPMD program: ntot 128-row tiles x [ncols]-col bank matmul + max."""
    nc = bass.Bass("TRN2", target_bir_lowering=False, debug=False,
                   num_devices=N_CORES)
    f8, f32 = mybir.dt.float8e4, mybir.dt.float32
    AX = mybir.AxisListType.X
    DR = mybir.MatmulPerfMode.DoubleRow
    fills = _fills(ncols)
    nf = len(fills)

    d_bank = nc.dram_tensor("bank", [128, ncols, 2], f8,
                            kind="ExternalInput").ap()
    d_fT = nc.dram_tensor("fT", [128, 2, ntot * 128], f8,
                          kind="ExternalInput").ap()
    d_out = nc.dram_tensor("negm", [128, ntot * nf], f32,
                           kind="ExternalOutput").ap()

    with tile.TileContext(nc) as tc:
        import contextlib
        with contextlib.ExitStack() as ctx:
            singles = ctx.enter_context(tc.tile_pool(name="singles", bufs=1))
            psum = ctx.enter_context(tc.tile_pool(name="psum", bufs=3,
                                                  space="PSUM"))
            wpsum = ctx.enter_context(tc.tile_pool(name="wpsum", bufs=1,
                                                   space="PSUM"))
            bank = singles.tile([128, ncols, 2], f8, tag="bank", name="bank")
            fT = singles.tile([128, 2, ntot * 128], f8, tag="fT", name="fT")
            outm = singles.tile([128, ntot * nf], f32, tag="outm", name="outm")

            # PE warmup during the DMA head: matmuls on a memset tile keep
            # the PE busy through one HAM activity window so the clock is
            # already at 2.4 GHz when the real matmuls start.
            wsrc = singles.tile([128, 2, 256], f8, tag="wsrc", name="wsrc")
            nc.gpsimd.memset(wsrc, 0.0)
            wps = wpsum.tile([128, 128], f32, tag="wps", name="wps")
            for _ in range(14):
                nc.tensor.matmul(wps, wsrc[:, :, 0:256:2],
                                 wsrc[:, :, 0:256:2], start=True, stop=True,
                                 perf_mode=DR)

            # input DMA, balanced across the three initiators' rings (the
            # aggregate is HBM-bandwidth-capped; balance so no ring drags)
            nc.sync.dma_start(out=bank[:, 0:1024, :], in_=d_bank[:, 0:1024, :])
            nc.scalar.dma_start(out=fT, in_=d_fT)
            if ncols > 1024:
                w = min(1024, ncols - 1024)
                nc.scalar.dma_start(out=bank[:, 1024:1024 + w, :],
                                    in_=d_bank[:, 1024:1024 + w, :])
            for c0 in range(2048, ncols, 1024):
                w = min(1024, ncols - c0)
                nc.gpsimd.dma_start(out=bank[:, c0:c0 + w, :],
                                    in_=d_bank[:, c0:c0 + w, :])

            for s in range(ntot):
                lhs = fT[:, :, s * 128:(s + 1) * 128]
                for fi, (c0, w) in enumerate(fills):
                    ps = psum.tile([128, FILL], f32, tag="ps", name="ps")
                    for h0 in range(0, w, 512):
                        hw = min(512, w - h0)
                        rhs = bank[:, c0 + h0:c0 + h0 + hw, :].rearrange(
                            "p c k -> p k c")
                        nc.tensor.matmul(ps[:, h0:h0 + hw], lhs, rhs,
                                         start=True, stop=True, perf_mode=DR)
                    nc.vector.reduce_max(
                        out=outm[:, s * nf + fi:s * nf + fi + 1],
                        in_=ps[:, 0:w:SUB], axis=AX)
                # per-slot output store: only the last slot's (tiny) DMA
                # completion sits in the critical path at drain time
                nc.scalar.dma_start(out=d_out[:, s * nf:(s + 1) * nf],
                                    in_=outm[:, s * nf:(s + 1) * nf])

    return nc


# ---------------------------------------------------------------------------
def kernel(output_feat1, output_feat2, pseudo_label1, pseudo_label2,
           pseudo_logits1, pseudo_logits2, output_ul1, output_ul2,
           selected_idx1, selected_idx2):
    f1 = np.ascontiguousarray(np.asarray(output_feat1, dtype=np.float32))
    f2 = np.ascontiguousarray(np.asarray(output_feat2, dtype=np.float32))
    pl1 = np.asarray(pseudo_label1).astype(np.int64)
    pl2 = np.asarray(pseudo_label2).astype(np.int64)
    pg1 = np.asarray(pseudo_logits1, dtype=np.float32)
    pg2 = np.asarray(pseudo_logits2, dtype=np.float32)
    ul1 = np.asarray(output_ul1, dtype=np.float32)
    ul2 = np.asarray(output_ul2, dtype=np.float32)
    idx1 = np.asarray(selected_idx1).astype(np.int64)
    idx2 = np.asarray(selected_idx2).astype(np.int64)

    b, c, h, w = ul1.shape
    ul1f = ul1.transpose(0, 2, 3, 1).reshape(-1, c)
    ul2f = ul2.transpose(0, 2, 3, 1).reshape(-1, c)
    memory = np.concatenate([ul1f[idx1], ul2f[idx2]], axis=0)     # [N, C]
    ml = np.concatenate([pl1[idx1], pl2[idx2]], axis=0)           # [N]

    pm = [((pg2 > POS_THRESH) & (pg1 < pg2)),
          ((pg1 > POS_THRESH) & (pg2 < pg1))]
    anchors = [f1, f2]
    alabels = [pl1, pl2]
    rows = [np.nonzero(m)[0] for m in pm]
    counts = [len(r) for r in rows]

    # device capacity: nt[d] 128-row tiles per core per direction; at most
    # HOST_CAP overflow rows per direction fall back to exact host compute
    nt = [max(1, -(-(max(cnt - HOST_CAP, 1)) // (128 * N_CORES)))
          for cnt in counts]
    cap = [t * 128 * N_CORES for t in nt]
    dev_rows = [r[:cp] for r, cp in zip(rows, cap)]
    host_rows = [r[cp:] for r, cp in zip(rows, cap)]
    ntot = nt[0] + nt[1]
    dev_cols = np.arange(0, N, COLSTEP)
    ncols = len(dev_cols)
    nf = len(_fills(ncols))

    # pos (score of the positive pair) for all masked rows, float64 on host
    pos_all = (f1.astype(np.float64) * f2.astype(np.float64)).sum(1) / TEMP

    # --- device inputs
    # bank [128, ncols, 2]: element (p, j, i) = channel i*128+p of column j
    bankT = memory[dev_cols].T.astype(np.float32)                 # [C, ncols]
    bank_dev = np.clip(bankT, -240, 240).reshape(2, 128, ncols)
    bank_dev = np.ascontiguousarray(bank_dev.transpose(1, 2, 0).astype(F8))
    # fT [128, 2, ntot*128]: element (p, i, r) = channel i*128+p of row r
    per_core = [t * 128 for t in nt]
    in_maps = []
    for core in range(N_CORES):
        cols = np.zeros((C, ntot * 128), dtype=np.float32)
        off = 0
        for d in range(2):
            sl = dev_rows[d][core * per_core[d]:(core + 1) * per_core[d]]
            if len(sl):
                cols[:, off:off + len(sl)] = anchors[d][sl].T / TEMP
            off += per_core[d]
        fTc = np.clip(cols, -240, 240).reshape(2, 128, ntot * 128)
        fTc = np.ascontiguousarray(fTc.transpose(1, 0, 2).astype(F8))
        in_maps.append({"bank": bank_dev, "fT": fTc})

    nc = _build_program(ntot, ncols)
    res = run_bass_kernel_spmd(nc, in_maps, list(range(N_CORES)))
    global LAST_RESULTS
    LAST_RESULTS = res

    # --- decode per-row approximate maxes
    mhat = [np.full(len(dev_rows[d]), -np.inf) for d in range(2)]
    for core in range(N_CORES):
        o = res.results[core]["negm"].astype(np.float64)   # [128, ntot*nf]
        for d in range(2):
            base_slot = 0 if d == 0 else nt[0]
            for t in range(nt[d]):
                g0 = core * per_core[d] + t * 128
                take = min(128, len(dev_rows[d]) - g0)
                if take <= 0:
                    continue
                sl = slice((base_slot + t) * nf, (base_slot + t + 1) * nf)
                mhat[d][g0:g0 + take] = o[:take, sl].max(axis=1)

    # --- host: exact contributions for selected + overflow rows, clamp rest
    CLAMP = -np.log(np.float64(EPS))
    mem64 = memory.astype(np.float64)
    total = np.float64(0)
    for d in range(2):
        sel = dev_rows[d][pos_all[dev_rows[d]] > mhat[d] - MARGIN]
        exact = np.concatenate([sel, host_rows[d]]).astype(np.int64)
        contrib = np.float64(0)
        if len(exact):
            A = anchors[d][exact].astype(np.float64)
            S = A @ mem64.T / TEMP                                # [k, N]
            pos_e = pos_all[exact]
            M = np.maximum(S.max(axis=1), pos_e)
            keep = (alabels[d][None, :] != ml[exact][:, None])
            Ssum = (np.exp(S - M[:, None]) * keep).sum(axis=1) \
                + np.exp(pos_e - M)
            lg = np.exp(pos_e - M) / (Ssum + EPS)
            contrib = (-np.log(lg + EPS)).sum()
        loss_d = (contrib + (counts[d] - len(exact)) * CLAMP) \
            / (counts[d] + 1e-12)
        total += loss_d
    return np.float32(total)
